# revision 50
# baseline (speedup 1.0000x reference)
"""GQA attention kernel for Trainium2, 8-core tensor-parallel.

Sharding: 8 cores = 2 batches x 4 KV-groups. Each core handles one
(batch, kv_group): projections for its 4 Q-heads + 1 KV-head, RoPE,
causal attention, and its row-shard of Wo -> partial [T, D] output.
Host sums the 4 partials per batch (the Wo all-reduce) at unshard.

fp16 matmul operands throughout (same PE rate as f32r at N>=256, half
the DMA/SBUF bytes, fast DVE modes). Attention runs in transposed
orientation (S^T tiles [s,q] from single K=128 matmuls). Softmax
row-sums accumulate on the vector engine into an SBUF fp16 tile,
reduced by ONE all-ones stationary matmul per (j,head) which also
broadcasts the sums across partitions for the O^T normalize. Diagonal
blocks are trimmed to the causal region at 128-col granularity; the
per-element causal mask is a single [128,128] triangle multiply per
diagonal block. exp() runs once per PAIR of s-chunks over a 2-bank
PSUM tile (halves the ACT fixed overhead). The B phase is otherwise
scalar(exp)-paced, so the previous q-tile's output projection blocks
are interleaved between attention heads as pure-tensor filler (and
j=1's K/V projection fills B(j=0)). Activations are host-pretransposed
(xC) so every input DMA is a contiguous 2D transfer. Output partials
are written fp16 (host sums in f32).

j=0 B-heads emit all S matmuls, then the j=1 filler, then all O
matmuls, so the first pair's exp latency hides behind tensor work; the
last tile's reciprocal+normalize are chunked 128-wide so the final
output projection starts on the first chunk.

Measured: ~232-237us HW (chip p-state varies +/-15% run-to-run) vs
356us baseline; tensor engine ~208us busy with ~3.6us in-span idle =
~97% of the 15 GF/core FLOP floor at fp16 peak. Remaining overhead is
the fixed ~13us NEFF preamble and ~12us epilogue semaphore drain.
"""
from contextlib import ExitStack

import numpy as np

import concourse.bass as bass
import concourse.mybir as mybir
import concourse.tile as tile
from concourse import bacc
from concourse.bass_utils import run_bass_kernel_spmd

B, T, D = 2, 2048, 2048
H, KV, HD = 16, 4, 128
R = H // KV                  # 4 query heads per kv head (per core)
GC = R * HD                  # 512 query-proj cols per core
THETA = 10000.0
TQ = 512                     # q-tile size
NJ = T // TQ                 # 4 q-tiles
ND = D // 128                # 16 contraction chunks
SCALE = float(HD) ** -0.5

F32 = mybir.dt.float32
F32R = mybir.dt.float32r
FP16 = mybir.dt.float16
AF = mybir.ActivationFunctionType

_CACHED_NC = None


def _build_nc():
    nc = bacc.Bacc("TRN2", target_bir_lowering=False, debug=False, num_devices=8)

    # xC: host-pretransposed activations, cols j*ND*TQ + d*TQ + c
    xC = nc.dram_tensor("xC", [128, NJ * ND * TQ], FP16, kind="ExternalInput").ap()
    wq = nc.dram_tensor("wq", [128, ND * GC], FP16, kind="ExternalInput").ap()
    wk = nc.dram_tensor("wk", [128, ND * HD], FP16, kind="ExternalInput").ap()
    wv = nc.dram_tensor("wv", [128, ND * HD], FP16, kind="ExternalInput").ap()
    wo = nc.dram_tensor("wo", [128, R * D], FP16, kind="ExternalInput").ap()
    cosT = nc.dram_tensor("cosT", [HD, T], FP16, kind="ExternalInput").ap()
    sinT = nc.dram_tensor("sinT", [HD, T], FP16, kind="ExternalInput").ap()
    trimask = nc.dram_tensor("trimask", [128, 128], FP16, kind="ExternalInput").ap()
    out = nc.dram_tensor("out", [T, D], FP16, kind="ExternalOutput").ap()

    with tile.TileContext(nc) as tc, ExitStack() as ctx:
        res = ctx.enter_context(tc.tile_pool(name="res", bufs=1))
        sb = ctx.enter_context(tc.tile_pool(name="sb", bufs=2))
        pp = ctx.enter_context(tc.tile_pool(name="pp", bufs=2, space="PSUM"))

        # ---- resident weights / tables ----
        # j=0 activations interleave with the weight DMAs in consumption
        # order so the first matmuls start as early as possible.
        # j=0 activations in 4 quarter DMAs (progressive arrival for A1/A2)
        wk_sb = res.tile([128, ND * HD], FP16)
        nc.sync.dma_start(wk_sb[:], wk[:])
        xt0_q = [sb.tile([128, 4 * TQ], FP16, tag="xtq", bufs=4, name=f"xt0_q{qtr}")
                 for qtr in range(4)]
        for qtr in range(4):
            nc.sync.dma_start(xt0_q[qtr][:],
                              xC[:, qtr * 4 * TQ:(qtr + 1) * 4 * TQ])
        # cos/sin/wv issue from the scalar queue (also HWDGE) in parallel
        # with the sync queue's wk/x issues
        cosj0 = sb.tile([128, TQ], FP16, tag="cos", bufs=2, name="cosj0")
        nc.scalar.dma_start(cosj0[:], cosT[:, 0:TQ])
        sinj0 = sb.tile([128, TQ], FP16, tag="sin", bufs=2, name="sinj0")
        nc.scalar.dma_start(sinj0[:], sinT[:, 0:TQ])
        wv_sb = res.tile([128, ND * HD], FP16)
        nc.scalar.dma_start(wv_sb[:], wv[:])
        wq_sb = res.tile([128, ND * GC], FP16)    # chunk d at cols [d*GC, (d+1)*GC)
        for part in range(4):
            nc.sync.dma_start(wq_sb[:, part * 4 * GC:(part + 1) * 4 * GC],
                              wq[:, part * 4 * GC:(part + 1) * 4 * GC])
        mask_sb = res.tile([128, 128], FP16)
        nc.sync.dma_start(mask_sb[:], trimask[:])
        wo_sb = res.tile([128, R * D], FP16)      # head h rows at cols [h*D, (h+1)*D)
        nc.sync.dma_start(wo_sb[:], wo[:])
        kT_sb = res.tile([128, T], FP16)          # K^T resident, filled per j
        v_sb = res.tile([128, T], FP16)           # V natural, chunk c at cols c*128
        ident = res.tile([128, 128], FP16)
        from concourse.masks import make_identity
        make_identity(nc, ident[:])
        ones_c = res.tile([128, 128], FP16)       # sigma-reduce+broadcast stationary
        nc.vector.memset(ones_c[:], 1.0)

        def c_block(o_t, q0p, qs, split_dma=False, scalar_only=False):
            # output projection for rows [q0p+qs*128, q0p+(qs+1)*128)
            # scalar_only: at the kernel tail the DVE queue is backed up
            # behind the rcb/oh chain; ob copies there must not ride it
            # (they free the pa PSUM ring for the next C matmul group).
            ob = sb.tile([128, D], FP16, tag="ob", bufs=3)
            for n in range(NJ):
                pc = pp.tile([128, 512], F32, tag="pa", bufs=2)
                for h2 in range(R):
                    nc.tensor.matmul(
                        pc[:], o_t[h2][:, qs * 128:(qs + 1) * 128],
                        wo_sb[:, h2 * D + n * 512:h2 * D + (n + 1) * 512],
                        start=(h2 == 0), stop=(h2 == R - 1))
                if scalar_only or n % 2 == 0:
                    nc.scalar.copy(ob[:, n * 512:(n + 1) * 512], pc[:])
                else:
                    nc.vector.tensor_copy(ob[:, n * 512:(n + 1) * 512], pc[:])
                if split_dma:
                    eng = nc.sync if n % 2 else nc.gpsimd
                    eng.dma_start(
                        out[q0p + qs * 128:q0p + (qs + 1) * 128,
                            n * 512:(n + 1) * 512],
                        ob[:, n * 512:(n + 1) * 512])
            if not split_dma:
                nc.gpsimd.dma_start(
                    out[q0p + qs * 128:q0p + (qs + 1) * 128, :], ob[:])

        def rope(dst, ps, cosj, sinj):
            # dst = ps * cos + rotate_half(ps) * sin   (partition dim = head dim)
            # one scalar op moves PSUM->SBUF fp16; the rest is 4x-mode DVE.
            ps_sb = sb.tile([128, TQ], FP16, tag="ps_sb", bufs=2)
            nc.scalar.copy(ps_sb[:], ps[:])
            rot = sb.tile([128, TQ], FP16, tag="rot", bufs=2)
            nc.vector.tensor_scalar_mul(rot[0:64, :], ps_sb[64:128, :], -1.0)
            nc.vector.tensor_copy(rot[64:128, :], ps_sb[0:64, :])
            tmp = sb.tile([128, TQ], FP16, tag="ropetmp", bufs=2)
            nc.vector.tensor_mul(tmp[:], rot[:], sinj[:])
            nc.vector.tensor_mul(dst, ps_sb[:], cosj[:])
            nc.vector.tensor_add(dst, dst, tmp[:])

        def stage(j):
            q0 = j * TQ
            if j == 0:
                return ([xt0_q[d // 4][:, (d % 4) * TQ:(d % 4 + 1) * TQ]
                         for d in range(ND)], cosj0, sinj0)
            xt_all = sb.tile([128, ND * TQ], FP16, tag="xt", bufs=2)
            nc.sync.dma_start(xt_all[:], xC[:, j * ND * TQ:(j + 1) * ND * TQ])
            xts = [xt_all[:, d * TQ:(d + 1) * TQ] for d in range(ND)]
            cosj = sb.tile([128, TQ], FP16, tag="cos", bufs=2)
            nc.sync.dma_start(cosj[:], cosT[:, q0:q0 + TQ])
            sinj = sb.tile([128, TQ], FP16, tag="sin", bufs=2)
            nc.sync.dma_start(sinj[:], sinT[:, q0:q0 + TQ])
            return xts, cosj, sinj

        def w_proj(ps, w_sb, xts, d0, d1):
            for d in range(d0, d1):
                nc.tensor.matmul(ps[:], w_sb[:, d * HD:(d + 1) * HD], xts[d],
                                 start=(d == 0), stop=(d == ND - 1))

        def a1_transposes(j, vt_sbt):
            for c4 in range(4):
                ptt = pp.tile([128, 128], FP16, tag="po", bufs=2)
                nc.tensor.transpose(ptt[:], vt_sbt[:, c4 * 128:(c4 + 1) * 128], ident[:])
                nc.vector.tensor_copy(
                    v_sb[:, (4 * j + c4) * 128:(4 * j + c4 + 1) * 128], ptt[:])

        def a1_finish(j, k_ps, vt_ps, cosj, sinj):
            rope(kT_sb[:, j * TQ:(j + 1) * TQ], k_ps, cosj, sinj)
            vt_sbt = sb.tile([128, TQ], FP16, tag="vtsb", bufs=2)
            nc.scalar.copy(vt_sbt[:], vt_ps[:])
            a1_transposes(j, vt_sbt)

        def a2_head(xts, h):
            q_ps = pp.tile([128, TQ], F32, tag="pa", bufs=2)
            for d in range(ND):
                nc.tensor.matmul(
                    q_ps[:], wq_sb[:, d * GC + h * 128:d * GC + (h + 1) * 128],
                    xts[d], start=(d == 0), stop=(d == ND - 1))
            return q_ps

        def a2_rope(q_ps, cosj, sinj):
            qh = sb.tile([128, TQ], FP16, tag="qsb", bufs=5)
            rope(qh[:], q_ps, cosj, sinj)
            return qh

        prev_o, prev_q0 = None, 0
        st1 = kv1 = None
        for j in range(NJ):
            q0 = j * TQ
            if j == 1:
                # staged during j=0; A1 and A2 heads 0-1 hoisted into B(0)
                xts, cosj, sinj = st1
                q_tiles = list(q1_tiles)
                for h in (2, 3):
                    q_tiles.append(a2_rope(a2_head(xts, h), cosj, sinj))
                a1_transposes(1, vt1_sbt_box[0])
            else:
                xts, cosj, sinj = stage(j)
                # ---- A1: K^T and V^T for s-tile j ----
                k_ps = pp.tile([128, TQ], F32, tag="pa", bufs=2)
                vt_ps = pp.tile([128, TQ], F32, tag="pa", bufs=2)
                w_proj(k_ps, wk_sb, xts, 0, ND)
                w_proj(vt_ps, wv_sb, xts, 0, ND)
                a1_finish(j, k_ps, vt_ps, cosj, sinj)
                # ---- A2: Q^T per head + rope ----
                q_tiles = [a2_rope(a2_head(xts, h), cosj, sinj) for h in range(R)]

            if j == 0:
                # stage j=1 now; its A1/A2 work becomes B(0)'s filler
                st1 = stage(1)
                kv1 = (pp.tile([128, TQ], F32, tag="pa", bufs=2, name="k_ps1"),
                       pp.tile([128, TQ], F32, tag="pa", bufs=2, name="vt_ps1"))
                q1_ps = []
                vt1_sbt_box = []

            # ---- B: causal attention per head (transposed S^T orientation),
            # interleaved with the previous tile's output projection so the
            # tensor engine has scalar-independent work during exp waits ----
            o_tiles = []
            nch = 4 * (j + 1)
            npair = nch // 2
            for h in range(R):
                o_ps = pp.tile([128, TQ], F32, tag="po", bufs=2)
                acc = sb.tile([128, TQ], FP16, tag="acc", bufs=2)
                if prev_o is None:
                    # j=0: emit j=1's A1/A2 matmuls inside the exp wait of
                    # this head's last pair (pure-tensor filler)
                    def filler0(h=h):
                        if h == 0:
                            w_proj(kv1[0], wk_sb, st1[0], 0, ND)
                        elif h == 1:
                            w_proj(kv1[1], wv_sb, st1[0], 0, ND)
                            rope(kT_sb[:, TQ:2 * TQ], kv1[0], st1[1], st1[2])
                            vt1_sbt_box.append(
                                sb.tile([128, TQ], FP16, tag="vtsb", bufs=2,
                                        name="vt1_sbt"))
                            nc.scalar.copy(vt1_sbt_box[0][:], kv1[1][:])
                        else:
                            q1_ps.append(a2_head(st1[0], h - 2))
                else:
                    filler0 = None
                o_defer = [] if prev_o is None else None
                for i in range(npair):
                    pair = ((0, 2 * i), (1, 2 * i + 1))
                    sp = pp.tile([128, 2 * TQ], F32, tag="sp", bufs=2)
                    p_sb = sb.tile([128, 2 * TQ], FP16, tag="psb", bufs=4)
                    for k, c in pair:
                        m = c - 4 * j
                        lo = m * 128 if m > 0 else 0
                        nc.tensor.matmul(sp[:, k * TQ + lo:(k + 1) * TQ],
                                         kT_sb[:, c * 128:(c + 1) * 128],
                                         q_tiles[h][:, lo:TQ], start=True, stop=True)
                    if pair[1][1] < 4 * j:  # both off-diagonal: one wide exp
                        nc.scalar.activation(p_sb[:], sp[:], AF.Exp, scale=SCALE)
                    else:
                        for k, c in pair:
                            m = c - 4 * j
                            lo = m * 128 if m > 0 else 0
                            nc.scalar.activation(p_sb[:, k * TQ + lo:(k + 1) * TQ],
                                                 sp[:, k * TQ + lo:(k + 1) * TQ],
                                                 AF.Exp, scale=SCALE)
                    for k, c in pair:
                        m = c - 4 * j
                        if m >= 0:  # diagonal block: triangle mask
                            blk = slice(k * TQ + m * 128, k * TQ + (m + 1) * 128)
                            nc.vector.tensor_mul(p_sb[:, blk], p_sb[:, blk], mask_sb[:])
                    # sigma: accumulate exp rows on DVE (partition-reduced later)
                    for k, c in pair:
                        m = c - 4 * j
                        lo = m * 128 if m > 0 else 0
                        if i == 0 and k == 0:
                            nc.vector.tensor_copy(acc[:], p_sb[:, 0:TQ])
                        else:
                            nc.vector.tensor_add(acc[:, lo:TQ], acc[:, lo:TQ],
                                                 p_sb[:, k * TQ + lo:(k + 1) * TQ])
                    if i == npair - 1:
                        if filler0 is not None:
                            filler0()
                        # sigma reduce+broadcast before the last O pair so the
                        # reciprocal overlaps the final O matmuls
                        sg_ps = pp.tile([128, TQ], F32, tag="po", bufs=2)
                        nc.tensor.matmul(sg_ps[:], ones_c[:], acc[:],
                                         start=True, stop=True)
                        rcb = sb.tile([128, TQ], F32, tag="rcb", bufs=2)
                        if j != NJ - 1:  # j3: rcb chunks interleave with oh
                            nc.vector.reciprocal_approx_fast(rcb[:], sg_ps[:])

                    def o_emit(pair=pair, p_sb=p_sb, i=i):
                        for k, c in pair:
                            m = c - 4 * j
                            lo = m * 128 if m > 0 else 0
                            nc.tensor.matmul(
                                o_ps[:, lo:TQ], v_sb[:, c * 128:(c + 1) * 128],
                                p_sb[:, k * TQ + lo:(k + 1) * TQ],
                                start=(i == 0 and k == 0),
                                stop=(i == npair - 1 and k == 1))
                    if o_defer is None:
                        o_emit()
                    else:
                        o_defer.append(o_emit)
                if o_defer is not None:
                    # j=0: all O matmuls emitted after S+filler so the first
                    # pair's exp latency hides behind tensor work
                    for f in o_defer:
                        f()
                oh = sb.tile([128, TQ], FP16, tag="osb", bufs=8)
                if j == NJ - 1:
                    # chunked reciprocal+normalize, interleaved: C(j3) blocks
                    # start on the first 128-col chunk of oh immediately
                    for qq in range(4):
                        s_ = slice(qq * 128, (qq + 1) * 128)
                        nc.vector.reciprocal_approx_fast(rcb[:, s_], sg_ps[:, s_])
                        nc.vector.tensor_mul(oh[:, s_], o_ps[:, s_], rcb[:, s_])
                else:
                    nc.vector.tensor_mul(oh[:], o_ps[:], rcb[:])
                o_tiles.append(oh)
                # C(j-1) output projection: pure-tensor filler between heads
                if prev_o is not None:
                    c_block(prev_o, prev_q0, h,
                            scalar_only=(j == NJ - 1 and h == R - 1))
            if j == 0:
                q1_tiles = [a2_rope(qp, st1[1], st1[2]) for qp in q1_ps]
            prev_o, prev_q0 = o_tiles, q0

        # ---- C for the last q-tile ----
        for qs in range(4):
            c_block(prev_o, prev_q0, qs, split_dma=(qs == 3), scalar_only=True)

    nc.compile()
    return nc


def _get_nc():
    global _CACHED_NC
    if _CACHED_NC is None:
        _CACHED_NC = _build_nc()
    return _CACHED_NC


def _rope_tables_T():
    inv_freq = (1.0 / (THETA ** (np.arange(0, HD, 2, dtype=np.float32) / HD))).astype(np.float32)
    pos = np.arange(T, dtype=np.float32)
    freqs = np.outer(pos, inv_freq).astype(np.float32)      # [T, HD/2]
    emb = np.concatenate([freqs, freqs], axis=-1)           # [T, HD]
    return (np.cos(emb).T.astype(np.float16).copy(),
            np.sin(emb).T.astype(np.float16).copy())        # [HD, T]


def _tri_mask():
    # keep col >= row within a 128x128 diagonal block
    i = np.arange(128)[:, None]
    jj = np.arange(128)[None, :]
    return (jj >= i).astype(np.float16)


def kernel(x, Wq, Wk, Wv, Wo, _trace=False):
    x = np.asarray(x, dtype=np.float32)
    Wq = np.asarray(Wq, dtype=np.float16)
    Wk = np.asarray(Wk, dtype=np.float16)
    Wv = np.asarray(Wv, dtype=np.float16)
    Wo = np.asarray(Wo, dtype=np.float16)

    cosT, sinT = _rope_tables_T()
    trimask = _tri_mask()
    in_maps = []
    for core in range(8):
        b, g = core // KV, core % KV
        def chunkT(w):  # [ND*128, C] -> [128, ND*C] with chunk d at cols [d*C,(d+1)*C)
            nd = w.shape[0] // 128
            return np.ascontiguousarray(
                w.reshape(nd, 128, -1).transpose(1, 0, 2).reshape(128, -1))
        # xC[p, j*ND*TQ + d*TQ + c] = x[b][j*TQ + c, d*128 + p]
        xc = (x[b].T.astype(np.float16)
              .reshape(ND, 128, NJ, TQ).transpose(1, 2, 0, 3)
              .reshape(128, NJ * ND * TQ))
        in_maps.append({
            "xC": np.ascontiguousarray(xc),
            "wq": chunkT(Wq[:, g * GC:(g + 1) * GC]),
            "wk": chunkT(Wk[:, g * HD:(g + 1) * HD]),
            "wv": chunkT(Wv[:, g * HD:(g + 1) * HD]),
            "wo": chunkT(Wo[g * GC:(g + 1) * GC, :]),
            "cosT": cosT, "sinT": sinT, "trimask": trimask,
        })

    nc = _get_nc()
    res = run_bass_kernel_spmd(nc, in_maps, core_ids=list(range(8)), trace=_trace)

    outp = np.zeros((B, T, D), dtype=np.float32)
    for core in range(8):
        b = core // KV
        outp[b] += res.results[core]["out"].astype(np.float32)
    if _trace:
        kernel._last_exec_time_ns = res.exec_time_ns
        kernel._last_trace = res.instructions_and_trace
    return outp


# revision 57
# speedup vs baseline: 1.0033x; 1.0033x over previous
"""GQA attention kernel for Trainium2, 8-core tensor-parallel.

Sharding: 8 cores = 2 batches x 4 KV-groups. Each core handles one
(batch, kv_group): projections for its 4 Q-heads + 1 KV-head, RoPE,
causal attention, and its row-shard of Wo -> partial [T, D] output.
Host sums the 4 partials per batch (the Wo all-reduce) at unshard.

fp16 matmul operands throughout (same PE rate as f32r at N>=256, half
the DMA/SBUF bytes, fast DVE modes). Attention runs in transposed
orientation (S^T tiles [s,q] from single K=128 matmuls). Softmax
row-sums accumulate on the vector engine into an SBUF fp16 tile,
reduced by ONE all-ones stationary matmul per (j,head) which also
broadcasts the sums across partitions for the O^T normalize. Diagonal
blocks are trimmed to the causal region at 128-col granularity; the
per-element causal mask is a single [128,128] triangle multiply per
diagonal block. exp() runs once per PAIR of s-chunks over a 2-bank
PSUM tile (halves the ACT fixed overhead). The B phase is otherwise
scalar(exp)-paced, so the previous q-tile's output projection blocks
are interleaved between attention heads as pure-tensor filler (and
j=1's K/V projection fills B(j=0)). Activations are host-pretransposed
(xC) so every input DMA is a contiguous 2D transfer. Output partials
are written fp16 (host sums in f32).

j=0 B-heads emit all S matmuls, then the j=1 filler, then all O
matmuls, so the first pair's exp latency hides behind tensor work; the
last tile's reciprocal+normalize are chunked 128-wide so the final
output projection starts on the first chunk.

Measured: ~232-237us HW (chip p-state varies +/-15% run-to-run) vs
356us baseline; tensor engine ~208us busy with ~3.6us in-span idle =
~97% of the 15 GF/core FLOP floor at fp16 peak. Remaining overhead is
the fixed ~13us NEFF preamble and ~12us epilogue semaphore drain.
"""
from contextlib import ExitStack

import numpy as np

import concourse.bass as bass
import concourse.mybir as mybir
import concourse.tile as tile
from concourse import bacc
from concourse.bass_utils import run_bass_kernel_spmd

B, T, D = 2, 2048, 2048
H, KV, HD = 16, 4, 128
R = H // KV                  # 4 query heads per kv head (per core)
GC = R * HD                  # 512 query-proj cols per core
THETA = 10000.0
TQ = 512                     # q-tile size
NJ = T // TQ                 # 4 q-tiles
ND = D // 128                # 16 contraction chunks
SCALE = float(HD) ** -0.5

F32 = mybir.dt.float32
F32R = mybir.dt.float32r
FP16 = mybir.dt.float16
AF = mybir.ActivationFunctionType

_CACHED_NC = None


def _build_nc():
    nc = bacc.Bacc("TRN2", target_bir_lowering=False, debug=False, num_devices=8)

    # xC: host-pretransposed activations, cols j*ND*TQ + d*TQ + c
    xC = nc.dram_tensor("xC", [128, NJ * ND * TQ], FP16, kind="ExternalInput").ap()
    wq = nc.dram_tensor("wq", [128, ND * GC], FP16, kind="ExternalInput").ap()
    wk = nc.dram_tensor("wk", [128, ND * HD], FP16, kind="ExternalInput").ap()
    wv = nc.dram_tensor("wv", [128, ND * HD], FP16, kind="ExternalInput").ap()
    wo = nc.dram_tensor("wo", [128, R * D], FP16, kind="ExternalInput").ap()
    cosT = nc.dram_tensor("cosT", [HD, T], FP16, kind="ExternalInput").ap()
    sinT = nc.dram_tensor("sinT", [HD, T], FP16, kind="ExternalInput").ap()
    trimask = nc.dram_tensor("trimask", [128, 128], FP16, kind="ExternalInput").ap()
    out = nc.dram_tensor("out", [T, D], FP16, kind="ExternalOutput").ap()

    with tile.TileContext(nc) as tc, ExitStack() as ctx:
        res = ctx.enter_context(tc.tile_pool(name="res", bufs=1))
        sb = ctx.enter_context(tc.tile_pool(name="sb", bufs=2))
        pp = ctx.enter_context(tc.tile_pool(name="pp", bufs=2, space="PSUM"))

        # ---- resident weights / tables ----
        # j=0 activations interleave with the weight DMAs in consumption
        # order so the first matmuls start as early as possible.
        # j=0 activations in 4 quarter DMAs (progressive arrival for A1/A2)
        wk_sb = res.tile([128, ND * HD], FP16)
        nc.sync.dma_start(wk_sb[:], wk[:])
        xt0_q = [sb.tile([128, 4 * TQ], FP16, tag="xtq", bufs=4, name=f"xt0_q{qtr}")
                 for qtr in range(4)]
        for qtr in range(4):
            nc.sync.dma_start(xt0_q[qtr][:],
                              xC[:, qtr * 4 * TQ:(qtr + 1) * 4 * TQ])
        # cos/sin/wv issue from the scalar queue (also HWDGE) in parallel
        # with the sync queue's wk/x issues
        cosj0 = sb.tile([128, TQ], FP16, tag="cos", bufs=2, name="cosj0")
        nc.scalar.dma_start(cosj0[:], cosT[:, 0:TQ])
        sinj0 = sb.tile([128, TQ], FP16, tag="sin", bufs=2, name="sinj0")
        nc.scalar.dma_start(sinj0[:], sinT[:, 0:TQ])
        wv_sb = res.tile([128, ND * HD], FP16)
        nc.scalar.dma_start(wv_sb[:], wv[:])
        wq_sb = res.tile([128, ND * GC], FP16)    # chunk d at cols [d*GC, (d+1)*GC)
        for part in range(4):
            nc.sync.dma_start(wq_sb[:, part * 4 * GC:(part + 1) * 4 * GC],
                              wq[:, part * 4 * GC:(part + 1) * 4 * GC])
        mask_sb = res.tile([128, 128], FP16)
        nc.sync.dma_start(mask_sb[:], trimask[:])
        wo_sb = res.tile([128, R * D], FP16)      # head h rows at cols [h*D, (h+1)*D)
        nc.sync.dma_start(wo_sb[:], wo[:])
        kT_sb = res.tile([128, T], FP16)          # K^T resident, filled per j
        v_sb = res.tile([128, T], FP16)           # V natural, chunk c at cols c*128
        ident = res.tile([128, 128], FP16)
        from concourse.masks import make_identity
        make_identity(nc, ident[:])
        ones_c = res.tile([128, 128], FP16)       # sigma-reduce+broadcast stationary
        nc.vector.memset(ones_c[:], 1.0)

        def c_block(o_t, q0p, qs, split_dma=False, scalar_only=False,
                    ob=None, ns=None, dma=True):
            # output projection for rows [q0p+qs*128, q0p+(qs+1)*128)
            # scalar_only: at the kernel tail the DVE queue is backed up
            # behind the rcb/oh chain; ob copies there must not ride it
            # (they free the pa PSUM ring for the next C matmul group).
            # ns/ob/dma allow emitting the block in halves (filler split).
            if ob is None:
                ob = sb.tile([128, D], FP16, tag="ob", bufs=3, name="ob")
            for n in (range(NJ) if ns is None else ns):
                pc = pp.tile([128, 512], F32, tag="pa", bufs=2)
                for h2 in range(R):
                    nc.tensor.matmul(
                        pc[:], o_t[h2][:, qs * 128:(qs + 1) * 128],
                        wo_sb[:, h2 * D + n * 512:h2 * D + (n + 1) * 512],
                        start=(h2 == 0), stop=(h2 == R - 1))
                if scalar_only or n % 2 == 0:
                    nc.scalar.copy(ob[:, n * 512:(n + 1) * 512], pc[:])
                else:
                    nc.vector.tensor_copy(ob[:, n * 512:(n + 1) * 512], pc[:])
                if split_dma:
                    eng = nc.sync if n % 2 else nc.gpsimd
                    eng.dma_start(
                        out[q0p + qs * 128:q0p + (qs + 1) * 128,
                            n * 512:(n + 1) * 512],
                        ob[:, n * 512:(n + 1) * 512])
            if dma and not split_dma:
                nc.gpsimd.dma_start(
                    out[q0p + qs * 128:q0p + (qs + 1) * 128, :], ob[:])
            return ob

        def rope(dst, ps, cosj, sinj):
            # dst = ps * cos + rotate_half(ps) * sin   (partition dim = head dim)
            # one scalar op moves PSUM->SBUF fp16; the rest is 4x-mode DVE.
            ps_sb = sb.tile([128, TQ], FP16, tag="ps_sb", bufs=2)
            nc.scalar.copy(ps_sb[:], ps[:])
            rot = sb.tile([128, TQ], FP16, tag="rot", bufs=2)
            nc.vector.tensor_scalar_mul(rot[0:64, :], ps_sb[64:128, :], -1.0)
            nc.vector.tensor_copy(rot[64:128, :], ps_sb[0:64, :])
            tmp = sb.tile([128, TQ], FP16, tag="ropetmp", bufs=2)
            nc.vector.tensor_mul(tmp[:], rot[:], sinj[:])
            nc.vector.tensor_mul(dst, ps_sb[:], cosj[:])
            nc.vector.tensor_add(dst, dst, tmp[:])

        def stage(j):
            q0 = j * TQ
            if j == 0:
                return ([xt0_q[d // 4][:, (d % 4) * TQ:(d % 4 + 1) * TQ]
                         for d in range(ND)], cosj0, sinj0)
            xt_all = sb.tile([128, ND * TQ], FP16, tag="xt", bufs=2)
            nc.sync.dma_start(xt_all[:], xC[:, j * ND * TQ:(j + 1) * ND * TQ])
            xts = [xt_all[:, d * TQ:(d + 1) * TQ] for d in range(ND)]
            cosj = sb.tile([128, TQ], FP16, tag="cos", bufs=2)
            nc.sync.dma_start(cosj[:], cosT[:, q0:q0 + TQ])
            sinj = sb.tile([128, TQ], FP16, tag="sin", bufs=2)
            nc.sync.dma_start(sinj[:], sinT[:, q0:q0 + TQ])
            return xts, cosj, sinj

        def w_proj(ps, w_sb, xts, d0, d1):
            for d in range(d0, d1):
                nc.tensor.matmul(ps[:], w_sb[:, d * HD:(d + 1) * HD], xts[d],
                                 start=(d == 0), stop=(d == ND - 1))

        def a1_transposes(j, vt_sbt):
            for c4 in range(4):
                ptt = pp.tile([128, 128], FP16, tag="po", bufs=2)
                nc.tensor.transpose(ptt[:], vt_sbt[:, c4 * 128:(c4 + 1) * 128], ident[:])
                nc.vector.tensor_copy(
                    v_sb[:, (4 * j + c4) * 128:(4 * j + c4 + 1) * 128], ptt[:])

        def a1_finish(j, k_ps, vt_ps, cosj, sinj):
            rope(kT_sb[:, j * TQ:(j + 1) * TQ], k_ps, cosj, sinj)
            vt_sbt = sb.tile([128, TQ], FP16, tag="vtsb", bufs=2)
            nc.scalar.copy(vt_sbt[:], vt_ps[:])
            a1_transposes(j, vt_sbt)

        def a2_head(xts, h):
            q_ps = pp.tile([128, TQ], F32, tag="pa", bufs=2)
            for d in range(ND):
                nc.tensor.matmul(
                    q_ps[:], wq_sb[:, d * GC + h * 128:d * GC + (h + 1) * 128],
                    xts[d], start=(d == 0), stop=(d == ND - 1))
            return q_ps

        def a2_rope(q_ps, cosj, sinj):
            qh = sb.tile([128, TQ], FP16, tag="qsb", bufs=5)
            rope(qh[:], q_ps, cosj, sinj)
            return qh

        prev_o, prev_q0 = None, 0
        st1 = kv1 = None
        for j in range(NJ):
            q0 = j * TQ
            if j == 1:
                # staged during j=0; A1 and A2 heads 0-1 hoisted into B(0)
                xts, cosj, sinj = st1
                q_tiles = list(q1_tiles)
                for h in (2, 3):
                    q_tiles.append(a2_rope(a2_head(xts, h), cosj, sinj))
                a1_transposes(1, vt1_sbt_box[0])
            else:
                xts, cosj, sinj = stage(j)
                # ---- A1: K^T and V^T for s-tile j ----
                k_ps = pp.tile([128, TQ], F32, tag="pa", bufs=2)
                vt_ps = pp.tile([128, TQ], F32, tag="pa", bufs=2)
                w_proj(k_ps, wk_sb, xts, 0, ND)
                w_proj(vt_ps, wv_sb, xts, 0, ND)
                a1_finish(j, k_ps, vt_ps, cosj, sinj)
                # ---- A2: Q^T per head + rope ----
                q_tiles = [a2_rope(a2_head(xts, h), cosj, sinj) for h in range(R)]

            if j == 0:
                # stage j=1 now; its A1/A2 work becomes B(0)'s filler
                st1 = stage(1)
                kv1 = (pp.tile([128, TQ], F32, tag="pa", bufs=2, name="k_ps1"),
                       pp.tile([128, TQ], F32, tag="pa", bufs=2, name="vt_ps1"))
                q1_ps = []
                vt1_sbt_box = []

            # ---- B: causal attention per head (transposed S^T orientation),
            # interleaved with the previous tile's output projection so the
            # tensor engine has scalar-independent work during exp waits ----
            o_tiles = []
            nch = 4 * (j + 1)
            npair = nch // 2
            for h in range(R):
                o_ps = pp.tile([128, TQ], F32, tag="po", bufs=2)
                acc = sb.tile([128, TQ], FP16, tag="acc", bufs=2)
                if prev_o is None:
                    # j=0: emit j=1's A1/A2 matmuls inside the exp wait of
                    # this head's last pair (pure-tensor filler)
                    def filler0(h=h):
                        if h == 0:
                            w_proj(kv1[0], wk_sb, st1[0], 0, ND)
                        elif h == 1:
                            w_proj(kv1[1], wv_sb, st1[0], 0, ND)
                            rope(kT_sb[:, TQ:2 * TQ], kv1[0], st1[1], st1[2])
                            vt1_sbt_box.append(
                                sb.tile([128, TQ], FP16, tag="vtsb", bufs=2,
                                        name="vt1_sbt"))
                            nc.scalar.copy(vt1_sbt_box[0][:], kv1[1][:])
                        else:
                            q1_ps.append(a2_head(st1[0], h - 2))
                else:
                    filler0 = None
                so_h = (j == NJ - 1 and h == R - 1)
                o_defer = [] if prev_o is None else None
                for i in range(npair):
                    pair = ((0, 2 * i), (1, 2 * i + 1))
                    sp = pp.tile([128, 2 * TQ], F32, tag="sp", bufs=2)
                    p_sb = sb.tile([128, 2 * TQ], FP16, tag="psb", bufs=4)
                    for k, c in pair:
                        m = c - 4 * j
                        lo = m * 128 if m > 0 else 0
                        nc.tensor.matmul(sp[:, k * TQ + lo:(k + 1) * TQ],
                                         kT_sb[:, c * 128:(c + 1) * 128],
                                         q_tiles[h][:, lo:TQ], start=True, stop=True)
                    if pair[1][1] < 4 * j:  # both off-diagonal: one wide exp
                        nc.scalar.activation(p_sb[:], sp[:], AF.Exp, scale=SCALE)
                    else:
                        for k, c in pair:
                            m = c - 4 * j
                            lo = m * 128 if m > 0 else 0
                            nc.scalar.activation(p_sb[:, k * TQ + lo:(k + 1) * TQ],
                                                 sp[:, k * TQ + lo:(k + 1) * TQ],
                                                 AF.Exp, scale=SCALE)
                    for k, c in pair:
                        m = c - 4 * j
                        if m >= 0:  # diagonal block: triangle mask
                            blk = slice(k * TQ + m * 128, k * TQ + (m + 1) * 128)
                            nc.vector.tensor_mul(p_sb[:, blk], p_sb[:, blk], mask_sb[:])
                    # sigma: accumulate exp rows on DVE (partition-reduced later)
                    for k, c in pair:
                        m = c - 4 * j
                        lo = m * 128 if m > 0 else 0
                        if i == 0 and k == 0:
                            nc.vector.tensor_copy(acc[:], p_sb[:, 0:TQ])
                        else:
                            nc.vector.tensor_add(acc[:, lo:TQ], acc[:, lo:TQ],
                                                 p_sb[:, k * TQ + lo:(k + 1) * TQ])
                    if i == npair - 1:
                        if filler0 is not None:
                            filler0()
                        elif prev_o is not None:
                            # first half of C(j-1,h) fills this pair's exp wait
                            ob_h = c_block(prev_o, prev_q0, h, ns=(0, 1),
                                           dma=False, scalar_only=so_h)
                        # sigma reduce+broadcast before the last O pair so the
                        # reciprocal overlaps the final O matmuls
                        sg_ps = pp.tile([128, TQ], F32, tag="po", bufs=2)
                        nc.tensor.matmul(sg_ps[:], ones_c[:], acc[:],
                                         start=True, stop=True)
                        rcb = sb.tile([128, TQ], F32, tag="rcb", bufs=2)
                        if j != NJ - 1:  # j3: rcb chunks interleave with oh
                            nc.vector.reciprocal_approx_fast(rcb[:], sg_ps[:])

                    def o_emit(pair=pair, p_sb=p_sb, i=i):
                        for k, c in pair:
                            m = c - 4 * j
                            lo = m * 128 if m > 0 else 0
                            nc.tensor.matmul(
                                o_ps[:, lo:TQ], v_sb[:, c * 128:(c + 1) * 128],
                                p_sb[:, k * TQ + lo:(k + 1) * TQ],
                                start=(i == 0 and k == 0),
                                stop=(i == npair - 1 and k == 1))
                    if o_defer is None:
                        o_emit()
                    else:
                        o_defer.append(o_emit)
                if o_defer is not None:
                    # j=0: all O matmuls emitted after S+filler so the first
                    # pair's exp latency hides behind tensor work
                    for f in o_defer:
                        f()
                oh = sb.tile([128, TQ], FP16, tag="osb", bufs=8)
                if j == NJ - 1:
                    # chunked reciprocal+normalize, interleaved: C(j3) blocks
                    # start on the first 128-col chunk of oh immediately
                    for qq in range(4):
                        s_ = slice(qq * 128, (qq + 1) * 128)
                        nc.vector.reciprocal_approx_fast(rcb[:, s_], sg_ps[:, s_])
                        nc.vector.tensor_mul(oh[:, s_], o_ps[:, s_], rcb[:, s_])
                else:
                    nc.vector.tensor_mul(oh[:], o_ps[:], rcb[:])
                o_tiles.append(oh)
                # second half of C(j-1,h)
                if prev_o is not None:
                    c_block(prev_o, prev_q0, h, ns=(2, 3), ob=ob_h,
                            scalar_only=so_h)
            if j == 0:
                q1_tiles = [a2_rope(qp, st1[1], st1[2]) for qp in q1_ps]
            prev_o, prev_q0 = o_tiles, q0

        # ---- C for the last q-tile ----
        for qs in range(4):
            c_block(prev_o, prev_q0, qs, split_dma=(qs == 3), scalar_only=True)

    nc.compile()
    return nc


def _get_nc():
    global _CACHED_NC
    if _CACHED_NC is None:
        _CACHED_NC = _build_nc()
    return _CACHED_NC


def _rope_tables_T():
    inv_freq = (1.0 / (THETA ** (np.arange(0, HD, 2, dtype=np.float32) / HD))).astype(np.float32)
    pos = np.arange(T, dtype=np.float32)
    freqs = np.outer(pos, inv_freq).astype(np.float32)      # [T, HD/2]
    emb = np.concatenate([freqs, freqs], axis=-1)           # [T, HD]
    return (np.cos(emb).T.astype(np.float16).copy(),
            np.sin(emb).T.astype(np.float16).copy())        # [HD, T]


def _tri_mask():
    # keep col >= row within a 128x128 diagonal block
    i = np.arange(128)[:, None]
    jj = np.arange(128)[None, :]
    return (jj >= i).astype(np.float16)


def kernel(x, Wq, Wk, Wv, Wo, _trace=False):
    x = np.asarray(x, dtype=np.float32)
    Wq = np.asarray(Wq, dtype=np.float16)
    Wk = np.asarray(Wk, dtype=np.float16)
    Wv = np.asarray(Wv, dtype=np.float16)
    Wo = np.asarray(Wo, dtype=np.float16)

    cosT, sinT = _rope_tables_T()
    trimask = _tri_mask()
    in_maps = []
    for core in range(8):
        b, g = core // KV, core % KV
        def chunkT(w):  # [ND*128, C] -> [128, ND*C] with chunk d at cols [d*C,(d+1)*C)
            nd = w.shape[0] // 128
            return np.ascontiguousarray(
                w.reshape(nd, 128, -1).transpose(1, 0, 2).reshape(128, -1))
        # xC[p, j*ND*TQ + d*TQ + c] = x[b][j*TQ + c, d*128 + p]
        xc = (x[b].T.astype(np.float16)
              .reshape(ND, 128, NJ, TQ).transpose(1, 2, 0, 3)
              .reshape(128, NJ * ND * TQ))
        in_maps.append({
            "xC": np.ascontiguousarray(xc),
            "wq": chunkT(Wq[:, g * GC:(g + 1) * GC]),
            "wk": chunkT(Wk[:, g * HD:(g + 1) * HD]),
            "wv": chunkT(Wv[:, g * HD:(g + 1) * HD]),
            "wo": chunkT(Wo[g * GC:(g + 1) * GC, :]),
            "cosT": cosT, "sinT": sinT, "trimask": trimask,
        })

    nc = _get_nc()
    res = run_bass_kernel_spmd(nc, in_maps, core_ids=list(range(8)), trace=_trace)

    outp = np.zeros((B, T, D), dtype=np.float32)
    for core in range(8):
        b = core // KV
        outp[b] += res.results[core]["out"].astype(np.float32)
    if _trace:
        kernel._last_exec_time_ns = res.exec_time_ns
        kernel._last_trace = res.instructions_and_trace
    return outp
